# revision 1
# baseline (speedup 1.0000x reference)
"""Trainium2 Bass kernel for nn_ConvolutionDMax (segment_reduce).

Computes, for a ragged batch of segments concatenated along dim 0 of x:
  for each window size w in (1,2,3):
      h_w = relu(conv1d_valid(x, conv_w{i}) + conv_b{i})     # over full stream
      pool_w[seg] = max over rows fully inside seg of h_w    # ragged segment max
  out = tanh(concat(pool_1, pool_2, pool_3) @ lin_w.T + lin_b)

Strategy (8 NeuronCores, SPMD single program):
  - bias+relu commute with max => device computes max over *raw* conv outputs
    (PSUM), then relu(bias + max) on the tiny pooled tensor.
  - Host re-deals segments so that all 8 cores share one identical layout
    template: for each distinct size s, every core gets ceil(n_s/8) slots of
    size s (missing ones zero-filled dummies, discarded on host). Same-size
    slots are contiguous, so the per-segment ragged max becomes a few batched
    strided TensorReduce ops per PSUM supertile.
  - x is passed feature-major ([128, L] per core) so conv = 6 accumulating
    128x128 matmuls streaming tokens along the PE free axis (bf16 in, fp32
    PSUM accumulate).
"""

import os
from collections import defaultdict

import ml_dtypes
import numpy as np

N_CORES = 8
C = 128          # feature dim (partition dim everywhere)
ST = 1024        # supertile positions (2 PSUM banks, fp32)
CHUNK_STS = 4    # supertiles per DMA chunk
HALO = 2         # extra x columns so window taps can read past the last slot
MM = 512         # max matmul free dim (fp32)

_PROGRAM_CACHE = {}
LAST_RESULTS = None  # BassKernelResults of the most recent run (for test.py)


# --------------------------------------------------------------------------
# Layout planning (pure python/numpy, no device deps)
# --------------------------------------------------------------------------

class _Plan:
    __slots__ = (
        "template", "assign", "slot_off", "sts", "chunks", "L", "nslot",
        "nslot2", "max_clen",
    )


def _build_plan(sizes: np.ndarray) -> _Plan:
    """Template layout shared by all cores + per-core slot assignment."""
    by_size = defaultdict(list)
    for i, s in enumerate(sizes.tolist()):
        by_size[int(s)].append(i)

    template = []                      # slot -> segment size
    assign = [[] for _ in range(N_CORES)]  # core -> slot -> orig idx or -1
    for s in sorted(by_size, reverse=True):
        idxs = by_size[s]
        m = -(-len(idxs) // N_CORES)
        for j in range(m):
            template.append(s)
            for c in range(N_CORES):
                k = j * N_CORES + c
                assign[c].append(idxs[k] if k < len(idxs) else -1)

    # Slot offsets and supertiles (whole slots, <= ST positions each).
    # Slots are padded to even width (zero x spacer) so every slot base and
    # stride is even (fp32r matmul + future bf16 2x-mode alignment).
    slot_off = []
    sts = []          # (base, length, runs); run = (loc_off, slot0, cnt, s)
    cur_slots = []    # (slot_idx, size) of current supertile
    cur_base = 0
    off = 0

    def close_st():
        nonlocal off
        if (off - cur_base) % 2:  # fp32r matmul needs even free dim
            off += 1
        length = off - cur_base
        runs = []
        for j, s in cur_slots:
            if runs and runs[-1][3] == s:
                lo, s0, cnt, _ = runs[-1]
                runs[-1] = (lo, s0, cnt + 1, s)
            else:
                runs.append((slot_off[j] - cur_base, j, 1, s))
        sts.append((cur_base, length, runs))

    for j, s in enumerate(template):
        s_pad = s + (s & 1)
        if cur_slots and (off - cur_base) + s_pad > ST:
            close_st()
            cur_base = off
            cur_slots = []
        slot_off.append(off)
        cur_slots.append((j, s))
        off += s_pad
    if cur_slots:
        close_st()

    # DMA chunks: groups of CHUNK_STS supertiles.
    chunks = []       # (base, clen, [st indices])
    for i0 in range(0, len(sts), CHUNK_STS):
        grp = list(range(i0, min(i0 + CHUNK_STS, len(sts))))
        base = sts[grp[0]][0]
        clen = sts[grp[-1]][0] + sts[grp[-1]][1] - base
        chunks.append((base, clen, grp))

    p = _Plan()
    p.template = template
    p.assign = assign
    p.slot_off = slot_off
    p.sts = sts
    p.chunks = chunks
    p.L = off
    p.nslot = len(template)
    p.nslot2 = p.nslot + (p.nslot & 1)  # even, for fp32r linear matmul
    p.max_clen = max(cl for _, cl, _ in chunks)
    return p


# --------------------------------------------------------------------------
# Bass program
# --------------------------------------------------------------------------

# weight column-block index in the packed [128, 6*128] conv weight tensor
_BLK = [[0], [1, 2], [3, 4, 5]]


def _build_program(plan: _Plan):
    import concourse.tile as tile
    from concourse import bacc, mybir

    F32 = mybir.dt.float32
    F32R = mybir.dt.float32r
    BF16 = mybir.dt.bfloat16
    AF = mybir.ActivationFunctionType

    nc = bacc.Bacc("TRN2", target_bir_lowering=False, debug=False,
                   num_devices=N_CORES)

    xt_d = nc.dram_tensor("xt", [C, plan.L + HALO], BF16, kind="ExternalInput")
    wc_d = nc.dram_tensor("wconv", [C, 6 * C], BF16, kind="ExternalInput")
    lt_d = nc.dram_tensor("lint", [C, 3 * C], F32R, kind="ExternalInput")
    bs_d = nc.dram_tensor("biases", [C, 4], F32, kind="ExternalInput")
    out_d = nc.dram_tensor("out", [C, plan.nslot2], F32, kind="ExternalOutput")

    with tile.TileContext(nc) as tc:
        with (
            tc.tile_pool(name="wp", bufs=1) as wp,
            tc.tile_pool(name="xp", bufs=3) as xp,
            tc.tile_pool(name="pp", bufs=1) as pp,
            tc.tile_pool(name="ps", bufs=1, space="PSUM") as ps,
        ):
            w_sb = wp.tile([C, 6 * C], BF16, tag="w")
            l_sb = wp.tile([C, 3 * C], F32R, tag="l")
            b_sb = wp.tile([C, 4], F32, tag="b")
            nc.sync.dma_start(w_sb[:], wc_d.ap())
            nc.sync.dma_start(l_sb[:], lt_d.ap())
            nc.sync.dma_start(b_sb[:], bs_d.ap())

            pooled = [pp.tile([C, plan.nslot2], F32, tag=f"pool{w}", name=f"pool{w}")
                      for w in range(3)]
            pooledr = [pp.tile([C, plan.nslot2], F32R, tag=f"poolr{w}", name=f"poolr{w}")
                       for w in range(3)]
            out_sb = pp.tile([C, plan.nslot2], F32, tag="osb", name="osb")
            if plan.nslot2 != plan.nslot:
                for w in range(3):
                    nc.vector.memset(pooled[w][:, plan.nslot :], 0.0)

            # streams 1,2: ACT copies PSUM -> SBUF fp32 scratch; DVE reduces
            # from SBUF (PSUM banks recycle at ACT speed, not DVE speed).
            for base, clen, st_ids in plan.chunks:
                xc = xp.tile([C, plan.max_clen + HALO], BF16, tag="x", name="xc")
                nc.sync.dma_start(
                    xc[:, : clen + HALO],
                    xt_d.ap()[:, base : base + clen + HALO],
                )
                for sti in st_ids:
                    st_base, st_len, runs = plan.sts[sti]
                    lo = st_base - base
                    for w in range(3):
                        pt = ps.tile([C, st_len], F32, tag=f"w{w}", name=f"ps{w}")
                        for p0 in range(0, st_len, MM):
                            p1 = min(p0 + MM, st_len)
                            for k in range(w + 1):
                                nc.tensor.matmul(
                                    pt[:, p0:p1],
                                    w_sb[:, _BLK[w][k] * C : (_BLK[w][k] + 1) * C],
                                    xc[:, lo + k + p0 : lo + k + p1],
                                    start=(k == 0),
                                    stop=(k == w),
                                )
                        if w > 0:
                            yt = pp.tile([C, ST], F32, tag=f"y{w}",
                                         name=f"y{w}", bufs=3)
                            nc.scalar.copy(yt[:, :st_len], pt[:])
                            red_src = yt
                        else:
                            red_src = pt
                        for loc_off, slot0, cnt, s in runs:
                            span = s - w  # s - (w+1) + 1
                            sp = s + (s & 1)
                            src = (
                                red_src[:, loc_off : loc_off + cnt * sp]
                                .rearrange("p (n s) -> p n s", s=sp)[:, :, :span]
                            )
                            nc.vector.tensor_reduce(
                                out=pooled[w][:, slot0 : slot0 + cnt],
                                in_=src,
                                axis=mybir.AxisListType.X,
                                op=mybir.AluOpType.max,
                            )

            for w in range(3):
                nc.scalar.activation(
                    pooledr[w][:], pooled[w][:], AF.Relu, bias=b_sb[:, w : w + 1]
                )

            for c0 in range(0, plan.nslot2, MM):
                c1 = min(c0 + MM, plan.nslot2)
                lp = ps.tile([C, c1 - c0], F32, tag="lin", name="lps")
                for w in range(3):
                    nc.tensor.matmul(
                        lp[:],
                        l_sb[:, w * C : (w + 1) * C],
                        pooledr[w][:, c0:c1],
                        start=(w == 0),
                        stop=(w == 2),
                    )
                nc.scalar.activation(
                    out_sb[:, c0:c1], lp[:], AF.Tanh, bias=b_sb[:, 3:4]
                )

            nc.sync.dma_start(out_d.ap(), out_sb[:])

    nc.compile()
    return nc


# --------------------------------------------------------------------------
# Host entry point
# --------------------------------------------------------------------------

def kernel(x, sizes, conv_w0, conv_b0, conv_w1, conv_b1, conv_w2, conv_b2,
           lin_w, lin_b):
    global LAST_RESULTS
    from concourse.bass_utils import run_bass_kernel_spmd

    x = np.asarray(x, np.float32)
    sizes = np.asarray(sizes, np.int32)
    convs = [
        (np.asarray(conv_w0, np.float32), np.asarray(conv_b0, np.float32)),
        (np.asarray(conv_w1, np.float32), np.asarray(conv_b1, np.float32)),
        (np.asarray(conv_w2, np.float32), np.asarray(conv_b2, np.float32)),
    ]
    lin_w = np.asarray(lin_w, np.float32)
    lin_b = np.asarray(lin_b, np.float32)

    plan = _build_plan(sizes)
    key = tuple(plan.template)
    if key not in _PROGRAM_CACHE:
        _PROGRAM_CACHE[key] = _build_program(plan)
    nc = _PROGRAM_CACHE[key]

    # Packed conv weights: block b = tap k of stream w, transposed to [C, M].
    wconv = np.empty((C, 6 * C), ml_dtypes.bfloat16)
    for w in range(3):
        cw, _ = convs[w]
        for k in range(w + 1):
            b = _BLK[w][k]
            wconv[:, b * C : (b + 1) * C] = cw[:, :, k].T
    lint = np.empty((C, 3 * C), np.float32)
    for w in range(3):
        lint[:, w * C : (w + 1) * C] = lin_w[:, w * C : (w + 1) * C].T
    biases = np.empty((C, 4), np.float32)
    for w in range(3):
        biases[:, w] = convs[w][1]
    biases[:, 3] = lin_b

    starts = np.cumsum(sizes) - sizes
    slot_off = np.asarray(plan.slot_off, np.int64)
    tmpl = np.asarray(plan.template, np.int64)

    in_maps = []
    for c in range(N_CORES):
        amap = np.asarray(plan.assign[c], np.int64)
        # column -> source row in x (or -1 for dummy/pad)
        col_src = np.full(plan.L + HALO, -1, np.int64)
        real = amap >= 0
        for j in np.nonzero(real)[0]:
            s = tmpl[j]
            o = slot_off[j]
            col_src[o : o + s] = np.arange(starts[amap[j]], starts[amap[j]] + s)
        xt = np.zeros((C, plan.L + HALO), ml_dtypes.bfloat16)
        valid = col_src >= 0
        xt[:, valid] = x[col_src[valid]].T
        in_maps.append({
            "xt": xt,
            "wconv": wconv,
            "lint": lint,
            "biases": biases,
        })

    res = run_bass_kernel_spmd(nc, in_maps, core_ids=list(range(N_CORES)))
    LAST_RESULTS = res

    out = np.empty((len(sizes), C), np.float32)
    for c in range(N_CORES):
        amap = np.asarray(plan.assign[c], np.int64)
        sel = amap >= 0
        out[amap[sel]] = res.results[c]["out"].T[sel]
    return out



# revision 15
# speedup vs baseline: 1.0967x; 1.0967x over previous
"""Trainium2 Bass kernel for nn_ConvolutionDMax (segment_reduce).

Computes, for a ragged batch of segments concatenated along dim 0 of x:
  for each window size w in (1,2,3):
      h_w = relu(conv1d_valid(x, conv_w{i}) + conv_b{i})     # over full stream
      pool_w[seg] = max over rows fully inside seg of h_w    # ragged segment max
  out = tanh(concat(pool_1, pool_2, pool_3) @ lin_w.T + lin_b)

Strategy (8 NeuronCores, SPMD single program):
  - bias+relu commute with max => device computes max over *raw* conv outputs
    (PSUM), then relu(bias + max) on the tiny pooled tensor.
  - Host re-deals segments so that all 8 cores share one identical layout
    template: for each distinct size s, every core gets ceil(n_s/8) slots of
    size s (missing ones zero-filled dummies, discarded on host). Same-size
    slots are contiguous, so the per-segment ragged max becomes a few batched
    strided reduce ops per PSUM supertile.
  - x is passed feature-major ([128, L] per core) so conv = 6 accumulating
    128x128 matmuls streaming tokens along the PE free axis (bf16 in, fp32
    PSUM accumulate).
  - The ragged max is spread across three engines so the DVE (whose
    tensor_reduce runs at 1 elem/cycle) is not the bottleneck:
      stream w=0: DVE tensor_reduce straight from PSUM.
      streams w=1,2: ACT copies PSUM->SBUF; GpSimd does a strided
        tensor_max halving pass (max of first/second half of each slot);
        DVE reduces the halved data.  A slice of supertiles keeps the
        full DVE reduce instead to balance GpSimd load.
"""

import os
from collections import defaultdict

import ml_dtypes
import numpy as np

N_CORES = 8
C = 128          # feature dim (partition dim everywhere)
ST = 1024        # supertile positions (2 PSUM banks, fp32)
CHUNK_STS = 4    # supertiles per DMA chunk
HALO = 2         # extra x columns so window taps can read past the last slot
MM = 512         # max matmul free dim (fp32 PSUM bank)
W0A_MOD = 5      # w0 uses route A (direct PSUM reduce) on W0A_NUM of
W0A_NUM = 3      # every W0A_MOD supertiles; w1/w2 always route E

_PROGRAM_CACHE = {}
LAST_RESULTS = None  # BassKernelResults of the most recent run (for test.py)


# --------------------------------------------------------------------------
# Layout planning (pure python/numpy, no device deps)
# --------------------------------------------------------------------------

class _Plan:
    __slots__ = (
        "template", "assign", "slot_off", "sts", "chunks", "L", "nslot",
        "nslot2", "max_clen",
    )


def _build_plan(sizes: np.ndarray) -> _Plan:
    """Template layout shared by all cores + per-core slot assignment."""
    by_size = defaultdict(list)
    for i, s in enumerate(sizes.tolist()):
        by_size[int(s)].append(i)

    template = []                      # slot -> segment size
    assign = [[] for _ in range(N_CORES)]  # core -> slot -> orig idx or -1
    for s in sorted(by_size, reverse=True):
        idxs = by_size[s]
        m = -(-len(idxs) // N_CORES)
        for j in range(m):
            template.append(s)
            for c in range(N_CORES):
                k = j * N_CORES + c
                assign[c].append(idxs[k] if k < len(idxs) else -1)

    # Slot offsets and supertiles (whole slots, <= ST positions each).
    # Slots are padded to even width (zero x spacer) so every slot base and
    # stride is even.
    slot_off = []
    sts = []          # (base, length, runs); run = (loc_off, slot0, cnt, s)
    cur_slots = []    # (slot_idx, size) of current supertile
    cur_base = 0
    off = 0

    def close_st():
        nonlocal off
        if (off - cur_base) % 2:
            off += 1
        length = off - cur_base
        runs = []
        for j, s in cur_slots:
            if runs and runs[-1][3] == s:
                lo, s0, cnt, _ = runs[-1]
                runs[-1] = (lo, s0, cnt + 1, s)
            else:
                runs.append((slot_off[j] - cur_base, j, 1, s))
        sts.append((cur_base, length, runs))

    for j, s in enumerate(template):
        s_pad = s + (s & 1)
        if cur_slots and (off - cur_base) + s_pad > ST:
            close_st()
            cur_base = off
            cur_slots = []
        slot_off.append(off)
        cur_slots.append((j, s))
        off += s_pad
    if cur_slots:
        close_st()

    # DMA chunks: groups of supertiles.  The first groups are small so
    # matmuls start as soon as possible after the first short DMA.
    chunks = []       # (base, clen, [st indices])
    group_sizes = [1, 1, 2]
    i0 = 0
    while i0 < len(sts):
        g = group_sizes.pop(0) if group_sizes else CHUNK_STS
        grp = list(range(i0, min(i0 + g, len(sts))))
        base = sts[grp[0]][0]
        clen = sts[grp[-1]][0] + sts[grp[-1]][1] - base
        chunks.append((base, clen, grp))
        i0 += g

    p = _Plan()
    p.template = template
    p.assign = assign
    p.slot_off = slot_off
    p.sts = sts
    p.chunks = chunks
    p.L = off
    p.nslot = len(template)
    p.nslot2 = p.nslot + (p.nslot & 1)  # even, for fp32r linear matmul
    p.max_clen = max(cl for _, cl, _ in chunks)
    return p


# --------------------------------------------------------------------------
# Bass program
# --------------------------------------------------------------------------

# weight column-block index in the packed [128, 6*128] conv weight tensor
_BLK = [[0], [1, 2], [3, 4, 5]]


def _build_program(plan: _Plan):
    import concourse.tile as tile
    from concourse import bacc, mybir

    F32 = mybir.dt.float32
    F32R = mybir.dt.float32r
    BF16 = mybir.dt.bfloat16
    F16 = mybir.dt.float16
    AF = mybir.ActivationFunctionType

    nc = bacc.Bacc("TRN2", target_bir_lowering=False, debug=False,
                   num_devices=N_CORES)

    xt_d = nc.dram_tensor("xt", [C, plan.L + HALO], BF16, kind="ExternalInput")
    wc_d = nc.dram_tensor("wconv", [C, 6 * C], BF16, kind="ExternalInput")
    lt_d = nc.dram_tensor("lint", [C, 3 * C], F32R, kind="ExternalInput")
    bs_d = nc.dram_tensor("biases", [C, 4], F32, kind="ExternalInput")
    out_d = nc.dram_tensor("out", [C, plan.nslot2], F32, kind="ExternalOutput")

    with tile.TileContext(nc) as tc:
        with (
            tc.tile_pool(name="wp", bufs=1) as wp,
            tc.tile_pool(name="xp", bufs=3) as xp,
            tc.tile_pool(name="pp", bufs=1) as pp,
            tc.tile_pool(name="yp", bufs=1) as yp,
            tc.tile_pool(name="ps", bufs=1, space="PSUM") as ps,
        ):
            w_sb = wp.tile([C, 6 * C], BF16, tag="w")
            l_sb = wp.tile([C, 3 * C], F32R, tag="l")
            b_sb = wp.tile([C, 4], F32, tag="b")
            nc.sync.dma_start(w_sb[:], wc_d.ap())
            nc.sync.dma_start(l_sb[:], lt_d.ap())
            nc.sync.dma_start(b_sb[:], bs_d.ap())

            NS2 = plan.nslot2
            pooled3 = pp.tile([C, 3 * NS2], F32, tag="pool3", name="pool3")
            pooledr = [pp.tile([C, NS2], F32R, tag=f"poolr{w}", name=f"poolr{w}")
                       for w in range(3)]
            out_sb = pp.tile([C, NS2], F32, tag="osb", name="osb")

            def pooled3_w(w):
                return pooled3[:, w * NS2 : (w + 1) * NS2]

            if plan.nslot2 != plan.nslot:
                for w in range(3):
                    nc.vector.memset(pooled3_w(w)[:, plan.nslot :], 0.0)

            def reduce_runs(dst, src, runs, w):
                """Per-run DVE reduce of a full supertile view."""
                for loc_off, slot0, cnt, s in runs:
                    span = s - w
                    sp = s + (s & 1)
                    v = (
                        src[:, loc_off : loc_off + cnt * sp]
                        .rearrange("p (n s) -> p n s", s=sp)[:, :, :span]
                    )
                    nc.vector.tensor_reduce(
                        out=dst[:, slot0 : slot0 + cnt],
                        in_=v,
                        axis=mybir.AxisListType.X,
                        op=mybir.AluOpType.max,
                    )

            # tail (relu + linear + tanh + out DMA) is emitted in column
            # chunks interleaved with the supertile loop: engine queues are
            # FIFO, so emitting it all at the end would serialize it after
            # every drain instruction.
            tail_bounds = list(range(0, plan.nslot2, MM))

            def emit_tail(c0):
                c1 = min(c0 + MM, plan.nslot2)
                for w in range(3):
                    nc.scalar.activation(
                        pooledr[w][:, c0:c1], pooled3_w(w)[:, c0:c1],
                        AF.Relu, bias=b_sb[:, w : w + 1],
                    )
                lp = ps.tile([C, c1 - c0], F32, tag="lin", name="lps")
                for w in range(3):
                    nc.tensor.matmul(
                        lp[:],
                        l_sb[:, w * C : (w + 1) * C],
                        pooledr[w][:, c0:c1],
                        start=(w == 0),
                        stop=(w == 2),
                    )
                nc.scalar.activation(
                    out_sb[:, c0:c1], lp[:], AF.Tanh, bias=b_sb[:, 3:4]
                )
                nc.sync.dma_start(out_d.ap()[:, c0:c1], out_sb[:, c0:c1])

            # first slot index of each supertile, to know when a tail
            # column chunk's inputs are complete
            st_first_slot = [
                min(r[1] for r in runs_) if runs_ else 0
                for _, _, runs_ in plan.sts
            ]

            gp_seq = 0
            for base, clen, st_ids in plan.chunks:
                xc = xp.tile([C, plan.max_clen + HALO], BF16, tag="x", name="xc")
                nc.sync.dma_start(
                    xc[:, : clen + HALO],
                    xt_d.ap()[:, base : base + clen + HALO],
                )
                for sti in st_ids:
                    st_base, st_len, runs = plan.sts[sti]
                    lo = st_base - base

                    # conv matmuls: tap-major within each stream so each
                    # loaded weight is used for both 512-column chunks
                    pt = [None, None, None]
                    for w in (1, 2, 0):
                        pt[w] = ps.tile(
                            [C, st_len], F32, tag=f"w{w}", name=f"ps{w}",
                            bufs=1,
                        )
                        chunks_p = list(range(0, st_len, MM))
                        for k in range(w + 1):
                            for p0 in chunks_p:
                                p1 = min(p0 + MM, st_len)
                                nc.tensor.matmul(
                                    pt[w][:, p0:p1],
                                    w_sb[:, _BLK[w][k] * C : (_BLK[w][k] + 1) * C],
                                    xc[:, lo + k + p0 : lo + k + p1],
                                    start=(k == 0),
                                    stop=(k == w),
                                )

                    # Drain.  w0 alternates between route A (DVE
                    # tensor_reduce straight from PSUM) and joining route E.
                    # Route E: ACT does one contiguous fp16 copy per stream
                    # into a stream-sliced image yt3 (draining PSUM), then
                    # DVE runs ONE 4D strided TT max per run (halving every
                    # slot of every E-stream at 2 results/cycle) and ONE 4D
                    # reduce per run over the packed halves.
                    # The b-half view needs a t-stride of ST-1 (spans shrink
                    # by 1 per stream); the AP is hand-adjusted for that.
                    gp_seq += 1
                    w0_a = (gp_seq % W0A_MOD) < W0A_NUM
                    t0 = 1 if w0_a else 0
                    tn = 3 - t0
                    yt3 = yp.tile([C, 3 * ST], F16, tag="y3", name="y3",
                                  bufs=3)
                    if w0_a:
                        reduce_runs(pooled3_w(0), pt[0], runs, 0)
                    for w in range(t0, 3):
                        nc.scalar.copy(
                            yt3[:, w * ST : w * ST + st_len], pt[w][:]
                        )
                    hb3 = yp.tile([C, 3 * 576], F16, tag="h3", name="h3",
                                  bufs=3)
                    hpos = 0
                    dve_tail = []
                    for loc_off, slot0, cnt, s in runs:
                        sp = s + (s & 1)
                        h = (s - t0 + 1) // 2
                        src = (
                            yt3[:, t0 * ST : 3 * ST]
                            .rearrange("p (t q) -> p t q", q=ST)
                            [:, :, loc_off : loc_off + cnt * sp]
                            .rearrange("p t (n s) -> p t n s", s=sp)
                        )
                        va = src[:, :, :, :h]
                        # b half: cols [span_w - h, span_w) per stream w;
                        # span_w = s - w, so consecutive streams shift left
                        # by one column -> t-stride ST - 1.
                        vb = src[:, :, :, s - t0 - h : s - t0].copy()
                        ap = list(vb.ap)
                        st_dim = ap[1]
                        vb.ap[1] = (ST - 1, st_dim[1])
                        vo = (
                            hb3[:, : tn * 576]
                            .rearrange("p (t q) -> p t q", q=576)
                            [:, :, hpos : hpos + cnt * h]
                            .rearrange("p t (n h) -> p t n h", h=h)
                        )
                        nc.vector.tensor_tensor(
                            out=vo, in0=va, in1=vb, op=mybir.AluOpType.max,
                        )
                        dve_tail.append((hpos, slot0, cnt, h))
                        hpos += cnt * h
                        hpos += hpos & 1
                    for hpos0, slot0, cnt, h in dve_tail:
                        nc.vector.tensor_reduce(
                            out=pooled3[:, t0 * NS2 : 3 * NS2]
                            .rearrange("p (t q) -> p t q", q=NS2)
                            [:, :, slot0 : slot0 + cnt],
                            in_=hb3[:, : tn * 576]
                            .rearrange("p (t q) -> p t q", q=576)
                            [:, :, hpos0 : hpos0 + cnt * h]
                            .rearrange("p t (n h) -> p t n h", h=h),
                            axis=mybir.AxisListType.X,
                            op=mybir.AluOpType.max,
                        )

                    slots_done = (
                        st_first_slot[sti + 1]
                        if sti + 1 < len(plan.sts) else plan.nslot2
                    )
                    while tail_bounds and (
                        min(tail_bounds[0] + MM, plan.nslot2) <= slots_done
                    ):
                        emit_tail(tail_bounds.pop(0))

            while tail_bounds:
                emit_tail(tail_bounds.pop(0))

    nc.compile()
    return nc


# --------------------------------------------------------------------------
# Host entry point
# --------------------------------------------------------------------------

def kernel(x, sizes, conv_w0, conv_b0, conv_w1, conv_b1, conv_w2, conv_b2,
           lin_w, lin_b):
    global LAST_RESULTS
    from concourse.bass_utils import run_bass_kernel_spmd

    x = np.asarray(x, np.float32)
    sizes = np.asarray(sizes, np.int32)
    convs = [
        (np.asarray(conv_w0, np.float32), np.asarray(conv_b0, np.float32)),
        (np.asarray(conv_w1, np.float32), np.asarray(conv_b1, np.float32)),
        (np.asarray(conv_w2, np.float32), np.asarray(conv_b2, np.float32)),
    ]
    lin_w = np.asarray(lin_w, np.float32)
    lin_b = np.asarray(lin_b, np.float32)

    plan = _build_plan(sizes)
    key = tuple(plan.template)
    if key not in _PROGRAM_CACHE:
        _PROGRAM_CACHE[key] = _build_program(plan)
    nc = _PROGRAM_CACHE[key]

    # Packed conv weights: block b = tap k of stream w, transposed to [C, M].
    wconv = np.empty((C, 6 * C), ml_dtypes.bfloat16)
    for w in range(3):
        cw, _ = convs[w]
        for k in range(w + 1):
            b = _BLK[w][k]
            wconv[:, b * C : (b + 1) * C] = cw[:, :, k].T
    lint = np.empty((C, 3 * C), np.float32)
    for w in range(3):
        lint[:, w * C : (w + 1) * C] = lin_w[:, w * C : (w + 1) * C].T
    biases = np.empty((C, 4), np.float32)
    for w in range(3):
        biases[:, w] = convs[w][1]
    biases[:, 3] = lin_b

    starts = np.cumsum(sizes) - sizes
    slot_off = np.asarray(plan.slot_off, np.int64)
    tmpl = np.asarray(plan.template, np.int64)

    in_maps = []
    for c in range(N_CORES):
        amap = np.asarray(plan.assign[c], np.int64)
        # column -> source row in x (or -1 for dummy/pad)
        col_src = np.full(plan.L + HALO, -1, np.int64)
        real = amap >= 0
        for j in np.nonzero(real)[0]:
            s = tmpl[j]
            o = slot_off[j]
            col_src[o : o + s] = np.arange(starts[amap[j]], starts[amap[j]] + s)
        xt = np.zeros((C, plan.L + HALO), ml_dtypes.bfloat16)
        valid = col_src >= 0
        xt[:, valid] = x[col_src[valid]].T
        in_maps.append({
            "xt": xt,
            "wconv": wconv,
            "lint": lint,
            "biases": biases,
        })

    res = run_bass_kernel_spmd(nc, in_maps, core_ids=list(range(N_CORES)))
    LAST_RESULTS = res

    out = np.empty((len(sizes), C), np.float32)
    for c in range(N_CORES):
        amap = np.asarray(plan.assign[c], np.int64)
        sel = amap >= 0
        out[amap[sel]] = res.results[c]["out"].T[sel]
    return out


# revision 16
# speedup vs baseline: 1.1171x; 1.0187x over previous
"""Trainium2 Bass kernel for nn_ConvolutionDMax (segment_reduce).

Computes, for a ragged batch of segments concatenated along dim 0 of x:
  for each window size w in (1,2,3):
      h_w = relu(conv1d_valid(x, conv_w{i}) + conv_b{i})     # over full stream
      pool_w[seg] = max over rows fully inside seg of h_w    # ragged segment max
  out = tanh(concat(pool_1, pool_2, pool_3) @ lin_w.T + lin_b)

Strategy (8 NeuronCores, SPMD single program):
  - bias+relu commute with max => device computes max over *raw* conv outputs
    (PSUM), then relu(bias + max) on the tiny pooled tensor.
  - Host re-deals segments so that all 8 cores share one identical layout
    template: for each distinct size s, every core gets ceil(n_s/8) slots of
    size s (missing ones zero-filled dummies, discarded on host). Same-size
    slots are contiguous, so the per-segment ragged max becomes a few batched
    strided reduce ops per PSUM supertile.
  - x is passed feature-major ([128, L] per core) so conv = 6 accumulating
    128x128 matmuls streaming tokens along the PE free axis (bf16 in, fp32
    PSUM accumulate).
  - The ragged max is spread across three engines so the DVE (whose
    tensor_reduce runs at 1 elem/cycle) is not the bottleneck:
      stream w=0: DVE tensor_reduce straight from PSUM.
      streams w=1,2: ACT copies PSUM->SBUF; GpSimd does a strided
        tensor_max halving pass (max of first/second half of each slot);
        DVE reduces the halved data.  A slice of supertiles keeps the
        full DVE reduce instead to balance GpSimd load.
"""

import os
from collections import defaultdict

import ml_dtypes
import numpy as np

N_CORES = 8
C = 128          # feature dim (partition dim everywhere)
ST = 1024        # supertile positions (2 PSUM banks, fp32)
CHUNK_STS = 4    # supertiles per DMA chunk
HALO = 2         # extra x columns so window taps can read past the last slot
MM = 512         # max matmul free dim (fp32 PSUM bank)
W0A_MOD = 5      # w0 uses route A (direct PSUM reduce) on W0A_NUM of
W0A_NUM = 2      # every W0A_MOD supertiles; w1/w2 always route E

_PROGRAM_CACHE = {}
LAST_RESULTS = None  # BassKernelResults of the most recent run (for test.py)


# --------------------------------------------------------------------------
# Layout planning (pure python/numpy, no device deps)
# --------------------------------------------------------------------------

class _Plan:
    __slots__ = (
        "template", "assign", "slot_off", "sts", "chunks", "L", "nslot",
        "nslot2", "max_clen",
    )


def _build_plan(sizes: np.ndarray) -> _Plan:
    """Template layout shared by all cores + per-core slot assignment."""
    by_size = defaultdict(list)
    for i, s in enumerate(sizes.tolist()):
        by_size[int(s)].append(i)

    template = []                      # slot -> segment size
    assign = [[] for _ in range(N_CORES)]  # core -> slot -> orig idx or -1
    for s in sorted(by_size, reverse=True):
        idxs = by_size[s]
        m = -(-len(idxs) // N_CORES)
        for j in range(m):
            template.append(s)
            for c in range(N_CORES):
                k = j * N_CORES + c
                assign[c].append(idxs[k] if k < len(idxs) else -1)

    # Slot offsets and supertiles (whole slots, <= ST positions each).
    # Slots are padded to even width (zero x spacer) so every slot base and
    # stride is even.
    slot_off = []
    sts = []          # (base, length, runs); run = (loc_off, slot0, cnt, s)
    cur_slots = []    # (slot_idx, size) of current supertile
    cur_base = 0
    off = 0

    def close_st():
        nonlocal off
        if (off - cur_base) % 2:
            off += 1
        length = off - cur_base
        runs = []
        for j, s in cur_slots:
            if runs and runs[-1][3] == s:
                lo, s0, cnt, _ = runs[-1]
                runs[-1] = (lo, s0, cnt + 1, s)
            else:
                runs.append((slot_off[j] - cur_base, j, 1, s))
        sts.append((cur_base, length, runs))

    for j, s in enumerate(template):
        s_pad = s + (s & 1)
        if cur_slots and (off - cur_base) + s_pad > ST:
            close_st()
            cur_base = off
            cur_slots = []
        slot_off.append(off)
        cur_slots.append((j, s))
        off += s_pad
    if cur_slots:
        close_st()

    # DMA chunks: groups of supertiles.  The first groups are small so
    # matmuls start as soon as possible after the first short DMA.
    chunks = []       # (base, clen, [st indices])
    n = len(sts)
    group_sizes = [1, 1, 2]
    mid = max(0, n - sum(group_sizes) - 4)
    plan_sizes = group_sizes + [CHUNK_STS] * (mid // CHUNK_STS) + [2, 1, 1]
    i0 = 0
    for g in plan_sizes:
        if i0 >= n:
            break
        grp = list(range(i0, min(i0 + g, n)))
        base = sts[grp[0]][0]
        clen = sts[grp[-1]][0] + sts[grp[-1]][1] - base
        chunks.append((base, clen, grp))
        i0 += g
    while i0 < n:
        grp = list(range(i0, min(i0 + CHUNK_STS, n)))
        base = sts[grp[0]][0]
        clen = sts[grp[-1]][0] + sts[grp[-1]][1] - base
        chunks.append((base, clen, grp))
        i0 += CHUNK_STS

    p = _Plan()
    p.template = template
    p.assign = assign
    p.slot_off = slot_off
    p.sts = sts
    p.chunks = chunks
    p.L = off
    p.nslot = len(template)
    p.nslot2 = p.nslot + (p.nslot & 1)  # even, for fp32r linear matmul
    p.max_clen = max(cl for _, cl, _ in chunks)
    return p


# --------------------------------------------------------------------------
# Bass program
# --------------------------------------------------------------------------

# weight column-block index in the packed [128, 6*128] conv weight tensor
_BLK = [[0], [1, 2], [3, 4, 5]]


def _build_program(plan: _Plan):
    import concourse.tile as tile
    from concourse import bacc, mybir

    F32 = mybir.dt.float32
    F32R = mybir.dt.float32r
    BF16 = mybir.dt.bfloat16
    F16 = mybir.dt.float16
    AF = mybir.ActivationFunctionType

    nc = bacc.Bacc("TRN2", target_bir_lowering=False, debug=False,
                   num_devices=N_CORES)

    xt_d = nc.dram_tensor("xt", [C, plan.L + HALO], BF16, kind="ExternalInput")
    wc_d = nc.dram_tensor("wconv", [C, 6 * C], BF16, kind="ExternalInput")
    lt_d = nc.dram_tensor("lint", [C, 3 * C], F32R, kind="ExternalInput")
    bs_d = nc.dram_tensor("biases", [C, 4], F32, kind="ExternalInput")
    out_d = nc.dram_tensor("out", [C, plan.nslot2], F32, kind="ExternalOutput")

    with tile.TileContext(nc) as tc:
        with (
            tc.tile_pool(name="wp", bufs=1) as wp,
            tc.tile_pool(name="xp", bufs=3) as xp,
            tc.tile_pool(name="pp", bufs=1) as pp,
            tc.tile_pool(name="yp", bufs=1) as yp,
            tc.tile_pool(name="ps", bufs=1, space="PSUM") as ps,
        ):
            w_sb = wp.tile([C, 6 * C], BF16, tag="w")
            l_sb = wp.tile([C, 3 * C], F32R, tag="l")
            b_sb = wp.tile([C, 4], F32, tag="b")
            nc.sync.dma_start(w_sb[:], wc_d.ap())
            nc.sync.dma_start(l_sb[:], lt_d.ap())
            nc.sync.dma_start(b_sb[:], bs_d.ap())

            NS2 = plan.nslot2
            pooled3 = pp.tile([C, 3 * NS2], F32, tag="pool3", name="pool3")
            pooledr = [pp.tile([C, NS2], F32R, tag=f"poolr{w}", name=f"poolr{w}")
                       for w in range(3)]
            out_sb = pp.tile([C, NS2], F32, tag="osb", name="osb")

            def pooled3_w(w):
                return pooled3[:, w * NS2 : (w + 1) * NS2]

            if plan.nslot2 != plan.nslot:
                for w in range(3):
                    nc.vector.memset(pooled3_w(w)[:, plan.nslot :], 0.0)

            def reduce_runs(dst, src, runs, w):
                """Per-run DVE reduce of a full supertile view."""
                for loc_off, slot0, cnt, s in runs:
                    span = s - w
                    sp = s + (s & 1)
                    v = (
                        src[:, loc_off : loc_off + cnt * sp]
                        .rearrange("p (n s) -> p n s", s=sp)[:, :, :span]
                    )
                    nc.vector.tensor_reduce(
                        out=dst[:, slot0 : slot0 + cnt],
                        in_=v,
                        axis=mybir.AxisListType.X,
                        op=mybir.AluOpType.max,
                    )

            # tail (relu + linear + tanh + out DMA) is emitted in column
            # chunks interleaved with the supertile loop: engine queues are
            # FIFO, so emitting it all at the end would serialize it after
            # every drain instruction.
            tail_bounds = list(range(0, plan.nslot2, MM))

            def emit_tail(c0):
                c1 = min(c0 + MM, plan.nslot2)
                for w in range(3):
                    nc.scalar.activation(
                        pooledr[w][:, c0:c1], pooled3_w(w)[:, c0:c1],
                        AF.Relu, bias=b_sb[:, w : w + 1],
                    )
                lp = ps.tile([C, c1 - c0], F32, tag="lin", name="lps")
                for w in range(3):
                    nc.tensor.matmul(
                        lp[:],
                        l_sb[:, w * C : (w + 1) * C],
                        pooledr[w][:, c0:c1],
                        start=(w == 0),
                        stop=(w == 2),
                    )
                nc.scalar.activation(
                    out_sb[:, c0:c1], lp[:], AF.Tanh, bias=b_sb[:, 3:4]
                )
                nc.sync.dma_start(out_d.ap()[:, c0:c1], out_sb[:, c0:c1])

            # first slot index of each supertile, to know when a tail
            # column chunk's inputs are complete
            st_first_slot = [
                min(r[1] for r in runs_) if runs_ else 0
                for _, _, runs_ in plan.sts
            ]

            gp_seq = 0
            for base, clen, st_ids in plan.chunks:
                xc = xp.tile([C, plan.max_clen + HALO], BF16, tag="x", name="xc")
                nc.sync.dma_start(
                    xc[:, : clen + HALO],
                    xt_d.ap()[:, base : base + clen + HALO],
                )
                for sti in st_ids:
                    st_base, st_len, runs = plan.sts[sti]
                    lo = st_base - base

                    # conv matmuls: tap-major within each stream so each
                    # loaded weight is used for both 512-column chunks
                    pt = [None, None, None]
                    for w in (1, 2, 0):
                        pt[w] = ps.tile(
                            [C, st_len], F32, tag=f"w{w}", name=f"ps{w}",
                            bufs=1,
                        )
                        chunks_p = list(range(0, st_len, MM))
                        for k in range(w + 1):
                            for p0 in chunks_p:
                                p1 = min(p0 + MM, st_len)
                                nc.tensor.matmul(
                                    pt[w][:, p0:p1],
                                    w_sb[:, _BLK[w][k] * C : (_BLK[w][k] + 1) * C],
                                    xc[:, lo + k + p0 : lo + k + p1],
                                    start=(k == 0),
                                    stop=(k == w),
                                )

                    # Drain.  w0 alternates between route A (DVE
                    # tensor_reduce straight from PSUM) and joining route E.
                    # Route E: ACT does one contiguous fp16 copy per stream
                    # into a stream-sliced image yt3 (draining PSUM), then
                    # DVE runs ONE 4D strided TT max per run (halving every
                    # slot of every E-stream at 2 results/cycle) and ONE 4D
                    # reduce per run over the packed halves.
                    # The b-half view needs a t-stride of ST-1 (spans shrink
                    # by 1 per stream); the AP is hand-adjusted for that.
                    gp_seq += 1
                    w0_a = (gp_seq % W0A_MOD) < W0A_NUM
                    t0 = 1 if w0_a else 0
                    tn = 3 - t0
                    yt3 = yp.tile([C, 3 * ST], F16, tag="y3", name="y3",
                                  bufs=3)
                    if w0_a:
                        reduce_runs(pooled3_w(0), pt[0], runs, 0)
                    for w in range(t0, 3):
                        nc.scalar.copy(
                            yt3[:, w * ST : w * ST + st_len], pt[w][:]
                        )
                    hb3 = yp.tile([C, 3 * 576], F16, tag="h3", name="h3",
                                  bufs=3)
                    hpos = 0
                    dve_tail = []
                    for loc_off, slot0, cnt, s in runs:
                        sp = s + (s & 1)
                        h = (s - t0 + 1) // 2
                        src = (
                            yt3[:, t0 * ST : 3 * ST]
                            .rearrange("p (t q) -> p t q", q=ST)
                            [:, :, loc_off : loc_off + cnt * sp]
                            .rearrange("p t (n s) -> p t n s", s=sp)
                        )
                        va = src[:, :, :, :h]
                        # b half: cols [span_w - h, span_w) per stream w;
                        # span_w = s - w, so consecutive streams shift left
                        # by one column -> t-stride ST - 1.
                        vb = src[:, :, :, s - t0 - h : s - t0].copy()
                        ap = list(vb.ap)
                        st_dim = ap[1]
                        vb.ap[1] = (ST - 1, st_dim[1])
                        vo = (
                            hb3[:, : tn * 576]
                            .rearrange("p (t q) -> p t q", q=576)
                            [:, :, hpos : hpos + cnt * h]
                            .rearrange("p t (n h) -> p t n h", h=h)
                        )
                        nc.vector.tensor_tensor(
                            out=vo, in0=va, in1=vb, op=mybir.AluOpType.max,
                        )
                        dve_tail.append((hpos, slot0, cnt, h))
                        hpos += cnt * h
                        hpos += hpos & 1
                    for hpos0, slot0, cnt, h in dve_tail:
                        nc.vector.tensor_reduce(
                            out=pooled3[:, t0 * NS2 : 3 * NS2]
                            .rearrange("p (t q) -> p t q", q=NS2)
                            [:, :, slot0 : slot0 + cnt],
                            in_=hb3[:, : tn * 576]
                            .rearrange("p (t q) -> p t q", q=576)
                            [:, :, hpos0 : hpos0 + cnt * h]
                            .rearrange("p t (n h) -> p t n h", h=h),
                            axis=mybir.AxisListType.X,
                            op=mybir.AluOpType.max,
                        )

                    slots_done = (
                        st_first_slot[sti + 1]
                        if sti + 1 < len(plan.sts) else plan.nslot2
                    )
                    while tail_bounds and (
                        min(tail_bounds[0] + MM, plan.nslot2) <= slots_done
                    ):
                        emit_tail(tail_bounds.pop(0))

            while tail_bounds:
                emit_tail(tail_bounds.pop(0))

    nc.compile()
    return nc


# --------------------------------------------------------------------------
# Host entry point
# --------------------------------------------------------------------------

def kernel(x, sizes, conv_w0, conv_b0, conv_w1, conv_b1, conv_w2, conv_b2,
           lin_w, lin_b):
    global LAST_RESULTS
    from concourse.bass_utils import run_bass_kernel_spmd

    x = np.asarray(x, np.float32)
    sizes = np.asarray(sizes, np.int32)
    convs = [
        (np.asarray(conv_w0, np.float32), np.asarray(conv_b0, np.float32)),
        (np.asarray(conv_w1, np.float32), np.asarray(conv_b1, np.float32)),
        (np.asarray(conv_w2, np.float32), np.asarray(conv_b2, np.float32)),
    ]
    lin_w = np.asarray(lin_w, np.float32)
    lin_b = np.asarray(lin_b, np.float32)

    plan = _build_plan(sizes)
    key = tuple(plan.template)
    if key not in _PROGRAM_CACHE:
        _PROGRAM_CACHE[key] = _build_program(plan)
    nc = _PROGRAM_CACHE[key]

    # Packed conv weights: block b = tap k of stream w, transposed to [C, M].
    wconv = np.empty((C, 6 * C), ml_dtypes.bfloat16)
    for w in range(3):
        cw, _ = convs[w]
        for k in range(w + 1):
            b = _BLK[w][k]
            wconv[:, b * C : (b + 1) * C] = cw[:, :, k].T
    lint = np.empty((C, 3 * C), np.float32)
    for w in range(3):
        lint[:, w * C : (w + 1) * C] = lin_w[:, w * C : (w + 1) * C].T
    biases = np.empty((C, 4), np.float32)
    for w in range(3):
        biases[:, w] = convs[w][1]
    biases[:, 3] = lin_b

    starts = np.cumsum(sizes) - sizes
    slot_off = np.asarray(plan.slot_off, np.int64)
    tmpl = np.asarray(plan.template, np.int64)

    in_maps = []
    for c in range(N_CORES):
        amap = np.asarray(plan.assign[c], np.int64)
        # column -> source row in x (or -1 for dummy/pad)
        col_src = np.full(plan.L + HALO, -1, np.int64)
        real = amap >= 0
        for j in np.nonzero(real)[0]:
            s = tmpl[j]
            o = slot_off[j]
            col_src[o : o + s] = np.arange(starts[amap[j]], starts[amap[j]] + s)
        xt = np.zeros((C, plan.L + HALO), ml_dtypes.bfloat16)
        valid = col_src >= 0
        xt[:, valid] = x[col_src[valid]].T
        in_maps.append({
            "xt": xt,
            "wconv": wconv,
            "lint": lint,
            "biases": biases,
        })

    res = run_bass_kernel_spmd(nc, in_maps, core_ids=list(range(N_CORES)))
    LAST_RESULTS = res

    out = np.empty((len(sizes), C), np.float32)
    for c in range(N_CORES):
        amap = np.asarray(plan.assign[c], np.int64)
        sel = amap >= 0
        out[amap[sel]] = res.results[c]["out"].T[sel]
    return out
